# revision 1
# baseline (speedup 1.0000x reference)
"""Performer (FAVOR+) linear attention on 8 TRN2 NeuronCores.

Sharding: core c handles batch b=c//4 and head group g=c%4 (4 of 16 heads).
Each core computes q/k/v projections for its heads from its batch's x,
runs the per-head softmax-kernel + linear-attention chain, and produces a
partial output projection y_c = o_heads @ Wo_slice.T.  Host sums the 4
partials per batch and adds the bias.

Matmuls use float32r (fp32 with 12 low mantissa bits dropped; 4x faster on
the PE at free-dim>=256).  End-to-end absmax error vs the fp32 reference is
~8e-4 relative to output scale (measured via numpy simulation of the
rounding).
"""
import sys
sys.path.insert(0, '/opt/trn_rl_repo')

import numpy as np
import concourse.bass as bass
import concourse.bacc as bacc
import concourse.tile as tile
from concourse import mybir
from concourse.bass_utils import run_bass_kernel_spmd

F32 = mybir.dt.float32
F32R = mybir.dt.float32r
AX = mybir.AxisListType.X
AF = mybir.ActivationFunctionType

B, N, D = 2, 4096, 1024
H, DH, M = 16, 64, 266            # heads, dim_head, nb_features
HPC = 4                           # heads per core
EPS = 1e-4
CNORM = DH ** -0.25               # data normalizer
RATIO = M ** -0.5
LNR = float(np.log(RATIO))
NT = N // 128                     # 32 n-tiles
# m-chunks over the 267-wide (ones col at 0, then 266 m cols) kp/qp tiles
MCH = [(0, 128), (128, 128), (256, 11)]  # (off, width)
LIMIT = "all"  # debug: p1 | k1 | k2 | q | all


def build():
    nc = bacc.Bacc("TRN2", target_bir_lowering=False, debug=False)

    xT = nc.dram_tensor("xT", [D, N], F32, kind="ExternalInput")
    wqT = nc.dram_tensor("wqT", [D, 256], F32, kind="ExternalInput")
    wkT = nc.dram_tensor("wkT", [D, 256], F32, kind="ExternalInput")
    wvT = nc.dram_tensor("wvT", [D, 256], F32, kind="ExternalInput")
    woP = nc.dram_tensor("woP", [128, 2048], F32, kind="ExternalInput")
    projc = nc.dram_tensor("projc", [DH, M], F32, kind="ExternalInput")
    ident = nc.dram_tensor("ident", [128, 128], F32, kind="ExternalInput")
    y = nc.dram_tensor("y", [N, D], F32, kind="ExternalOutput")
    qkv_d = nc.dram_tensor("qkv_scr", [3, HPC, N, DH], F32, kind="Internal")

    with tile.TileContext(nc) as tc:
        ctx_mgr = tc.tile_pool(name="const", bufs=1)
        with ctx_mgr as cpool, \
             tc.tile_pool(name="stage", bufs=1) as stg, \
             tc.tile_pool(name="stream", bufs=3) as strm, \
             tc.tile_pool(name="big", bufs=2) as big, \
             tc.tile_pool(name="small", bufs=4) as sml, \
             tc.tile_pool(name="ot", bufs=1) as otp_pool, \
             tc.tile_pool(name="ps", bufs=2, space="PSUM") as ps, \
             tc.tile_pool(name="psc", bufs=1, space="PSUM") as psc, \
             tc.tile_pool(name="pst", bufs=1, space="PSUM") as pst:

            # ---- constants / weights ----
            wq_r = cpool.tile([128, 8, 256], F32R, tag="wq")
            wk_r = cpool.tile([128, 8, 256], F32R, tag="wk")
            wv_r = cpool.tile([128, 8, 256], F32R, tag="wv")
            wo_r = cpool.tile([128, 2048], F32R, tag="wo")
            projc_r = cpool.tile([DH, M], F32R, tag="pj")
            ident_f = cpool.tile([128, 128], F32, tag="idf")
            ident_r = cpool.tile([128, 128], F32R, tag="idr")
            ones1_f = cpool.tile([1, 128], F32, tag="o1f")
            ones1_r = cpool.tile([1, 128], F32R, tag="o1r")
            ones128 = cpool.tile([128, 1], F32R, tag="o128")

            for dst, src in ((wq_r, wqT), (wk_r, wkT), (wv_r, wvT)):
                st = stg.tile([128, 8, 256], F32, tag="wstage")
                nc.sync.dma_start(st[:], src.ap().rearrange("(c p) n -> p c n", p=128))
                nc.vector.tensor_copy(dst[:], st[:])
            st = stg.tile([128, 2048], F32, tag="wstage")
            nc.sync.dma_start(st[:], woP.ap())
            nc.vector.tensor_copy(wo_r[:], st[:])
            st = stg.tile([DH, M], F32, tag="pstage")
            nc.sync.dma_start(st[:], projc.ap())
            nc.vector.tensor_copy(projc_r[:], st[:])
            nc.sync.dma_start(ident_f[:], ident.ap())
            nc.scalar.copy(ident_r[:], ident_f[:])
            nc.vector.memset(ones1_f[:], 1.0)
            nc.scalar.copy(ones1_r[:], ones1_f[:])
            nc.scalar.activation(ones128[:], ident_f[:, 0:1], AF.Identity,
                                 bias=1.0, scale=0.0)

            # ---- phase 1: QKV projections, spilled to DRAM scratch ----
            for j in range(NT):
                xt = strm.tile([128, 8, 128], F32, tag="xt", bufs=2)
                nc.sync.dma_start(
                    xt[:], xT.ap().rearrange("(c p) n -> p c n", p=128)[:, :, j*128:(j+1)*128])
                xt_r = strm.tile([128, 8, 128], F32R, tag="xtr", bufs=2)
                nc.vector.tensor_copy(xt_r[:], xt[:])
                for ti, w_r in ((0, wq_r), (1, wk_r), (2, wv_r)):
                    acc = ps.tile([128, 256], F32, tag="b256")
                    for dchunk in range(8):
                        nc.tensor.matmul(acc[:], xt_r[:, dchunk, :], w_r[:, dchunk, :],
                                         start=(dchunk == 0), stop=(dchunk == 7))
                    sb = strm.tile([128, 4, DH], F32, tag=f"qkv{ti}", bufs=2)
                    nc.vector.tensor_copy(sb[:], acc[:].rearrange("p (h e) -> p h e", e=DH))
                    nc.sync.dma_start(
                        qkv_d.ap()[ti].rearrange("h (j p) e -> j p h e", p=128)[j], sb[:])

            # ---- per-head chain ----
            nheads = 0 if LIMIT == "p1" else (1 if LIMIT in ("k1", "k2", "q") else HPC)  # "heads": 4 heads, no P3
            for h in range(nheads):
                kb = big.tile([128, NT, DH], F32, tag="hb", bufs=3, name="kb")
                nc.sync.dma_start(
                    kb[:], qkv_d.ap()[1, h].rearrange("(j p) e -> p j e", p=128))
                dashk = big.tile([128, NT, M], F32, tag="dashk", bufs=1)
                rmaxb = sml.tile([128, NT], F32, tag="rmaxb", bufs=2)
                diagk = sml.tile([128, NT], F32, tag="diagk", bufs=2)

                # K1: dash_k tiles, running stats
                for j in range(NT):
                    sqj = strm.tile([128, DH], F32, tag="sqj")
                    nc.scalar.activation(sqj[:], kb[:, j, :], AF.Square,
                                         accum_out=diagk[:, j:j+1])
                    ktp = pst.tile([DH, 128], F32, tag="tp64")
                    nc.tensor.transpose(ktp[:], kb[:, j, :], ident_f[:])
                    kt = strm.tile([DH, 128], F32R, tag="kt")
                    nc.scalar.copy(kt[:], ktp[:])
                    dash = ps.tile([128, M], F32, tag="b256")
                    nc.tensor.matmul(dash[:], kt[:], projc_r[:], start=True, stop=True)
                    nc.vector.tensor_copy(dashk[:, j, :], dash[:])
                    if j % 4 == 3:
                        nc.vector.reduce_max(rmaxb[:, j-3:j+1],
                                             dashk[:, j-3:j+1, :], axis=AX)

                if LIMIT == "k1":
                    continue
                # global max -> per-partition broadcast mkb
                gmax = sml.tile([128, 1], F32, tag="gmax")
                nc.vector.reduce_max(gmax[:], rmaxb[:], axis=AX)
                gm_ps = pst.tile([1, 128], F32, tag="tp64")
                nc.tensor.transpose(gm_ps[:], gmax[:], ident_f[:])
                gmrow = sml.tile([1, 128], F32, tag="gmrow")
                nc.vector.tensor_copy(gmrow[:], gm_ps[:])
                mk = sml.tile([1, 1], F32, tag="mk")
                nc.vector.reduce_max(mk[:], gmrow[:], axis=AX)
                mk_ps = pst.tile([128, 1], F32, tag="tp64")
                nc.tensor.matmul(mk_ps[:], ones1_f[:], mk[:], start=True, stop=True)
                mkl = sml.tile([128, 1], F32, tag="mkl")
                # mkl = lnr - mk
                nc.vector.tensor_scalar(mkl[:], mk_ps[:], -1.0, LNR,
                                        op0=mybir.AluOpType.mult, op1=mybir.AluOpType.add)

                # batched per-tile exp bias: -0.5c^2*diag + (lnr - mk)
                biaskb = sml.tile([128, NT], F32, tag="biaskb", bufs=2)
                nc.vector.tensor_scalar(biaskb[:], diagk[:],
                                        -0.5 * CNORM * CNORM, mkl[:],
                                        op0=mybir.AluOpType.mult,
                                        op1=mybir.AluOpType.add)

                # K2: kp = exp(dash - 0.5c^2*diag - mk + lnr), context accumulation
                vb = big.tile([128, NT, DH], F32, tag="hb", bufs=3, name="vb")
                nc.sync.dma_start(
                    vb[:], qkv_d.ap()[2, h].rearrange("(j p) e -> p j e", p=128))
                vxb = big.tile([128, NT, 66], F32R, tag="vxb", bufs=2)
                nc.scalar.copy(vxb[:, :, 0:DH], vb[:])
                nc.scalar.activation(vxb[:, :, DH:66], vb[:, :, 0:2], AF.Identity,
                                     bias=1.0, scale=0.0)
                ctx_ps = [psc.tile([128, 66], F32, tag=f"ctx{mc}", name=f"ctxp{mc}") for mc in range(2)]
                ctx_ps.append(psc.tile([11, 66], F32, tag="ctx2", name="ctxp2"))
                colsum_ps = pst.tile([1, 66], F32, tag="oe", name="colsum_ps")
                for j in range(NT):
                    # kp col 0 is a ones column (for k_cumsum); cols 1..266 = kp
                    kp = strm.tile([128, M + 1], F32R, tag="kp")
                    nc.scalar.activation(kp[:, 1:M+1], dashk[:, j, :], AF.Exp,
                                         bias=biaskb[:, j:j+1], scale=1.0)
                    nc.scalar.activation(kp[:, 0:1], kp[:, 1:2], AF.Identity,
                                         bias=1.0, scale=0.0)
                    for mc, (off, w) in enumerate(MCH):
                        nc.tensor.matmul(ctx_ps[mc][:], kp[:, off:off+w],
                                         vxb[:, j, :], start=(j == 0), stop=False)
                    # colsum_v accumulator (own group so it can be read while
                    # the ctx groups are still open)
                    nc.tensor.matmul(colsum_ps[:], kp[:, 0:1], vxb[:, j, :],
                                     start=(j == 0), stop=(j == NT - 1))

                # eps-correction: ctx += ratio*eps * ones x colsum_v.  Row 0 of
                # chunk0 (the kp-ones-column product) gets corrupted by the
                # correction; it is overwritten with the qp-side eps row below.
                colsum = sml.tile([1, 66], F32R, tag="colsum")
                nc.scalar.mul(colsum[:], colsum_ps[:], RATIO * EPS)
                for mc, (off, w) in enumerate(MCH):
                    nc.tensor.matmul(ctx_ps[mc][:], ones1_r[:, 0:w],
                                     colsum[:], start=False, stop=True)
                ctx_s = [big.tile([128, 66], F32R, tag=f"ctxs{mc}", name=f"ctxs{mc}") for mc in range(2)]
                ctx_s.append(big.tile([11, 66], F32R, tag="ctxs2", name="ctxs2"))
                for mc in range(3):
                    nc.scalar.copy(ctx_s[mc][:], ctx_ps[mc][:])
                # sum of corrected ctx over all rows, minus the (corrected)
                # row 0, = sum over real m rows
                smc_ps = pst.tile([1, 66], F32, tag="tp64")
                for mc, (off, w) in enumerate(MCH):
                    nc.tensor.matmul(smc_ps[:], ones128[0:w, :], ctx_s[mc][:],
                                     start=(mc == 0), stop=(mc == 2))
                smc_s = sml.tile([1, 66], F32, tag="smcs")
                nc.vector.tensor_copy(smc_s[:], smc_ps[:])
                eps_t = sml.tile([1, 66], F32, tag="epst")
                nc.vector.tensor_sub(eps_t[:], smc_s[:], ctx_s[0][0:1, :].bitcast(F32))
                # ctx chunk0 row 0 := ratio*eps*sumctx (consumed by qp ones col)
                nc.scalar.mul(ctx_s[0][0:1, :], eps_t[:], RATIO * EPS)

                if LIMIT == "k2":
                    continue
                # Q pass
                qb = big.tile([128, NT, DH], F32, tag="hb", bufs=3, name="qb")
                nc.sync.dma_start(
                    qb[:], qkv_d.ap()[0, h].rearrange("(j p) e -> p j e", p=128))
                for j in range(NT):
                    diagq = sml.tile([128, 1], F32, tag="diagq")
                    sqj = strm.tile([128, DH], F32, tag="sqj")
                    nc.scalar.activation(sqj[:], qb[:, j, :], AF.Square,
                                         accum_out=diagq[:])
                    qtp = pst.tile([DH, 128], F32, tag="tp64")
                    nc.tensor.transpose(qtp[:], qb[:, j, :], ident_f[:])
                    qt = strm.tile([DH, 128], F32R, tag="kt")
                    nc.scalar.copy(qt[:], qtp[:])
                    dash = ps.tile([128, M], F32, tag="b256")
                    nc.tensor.matmul(dash[:], qt[:], projc_r[:], start=True, stop=True)
                    rmaxq = sml.tile([128, 1], F32, tag="rmaxq")
                    nc.vector.reduce_max(rmaxq[:], dash[:], axis=AX)
                    biasq = sml.tile([128, 1], F32, tag="biasq")
                    nc.vector.tensor_scalar(biasq[:], diagq[:],
                                            -0.5 * CNORM * CNORM, LNR,
                                            op0=mybir.AluOpType.mult,
                                            op1=mybir.AluOpType.add)
                    nc.vector.tensor_sub(biasq[:], biasq[:], rmaxq[:])
                    qp = strm.tile([128, M + 1], F32R, tag="qp")
                    nc.scalar.activation(qp[:, 1:M+1], dash[:], AF.Exp,
                                         bias=biasq[:], scale=1.0)
                    nc.scalar.activation(qp[:, 0:1], qp[:, 1:2], AF.Identity,
                                         bias=1.0, scale=0.0)
                    qpt_ps = ps.tile([128, 384], F32R, tag="qpt", bufs=1)
                    for mc, (off, w) in enumerate(MCH):
                        nc.tensor.transpose(qpt_ps[0:w, mc*128:mc*128+128],
                                            qp[:, off:off+w], ident_r[:])
                    qpt = strm.tile([128, 384], F32R, tag="qpts")
                    nc.vector.tensor_copy(qpt[:, 0:256], qpt_ps[:, 0:256])
                    nc.vector.tensor_copy(qpt[0:11, 256:384], qpt_ps[0:11, 256:384])
                    oe_ps = pst.tile([128, 66], F32, tag="oe")
                    for mc, (off, w) in enumerate(MCH):
                        nc.tensor.matmul(oe_ps[:], qpt[0:w, mc*128:mc*128+128],
                                         ctx_s[mc][:], start=(mc == 0), stop=(mc == 2))
                    oe = strm.tile([128, 66], F32, tag="oes")
                    nc.scalar.copy(oe[:], oe_ps[:])
                    dinv = sml.tile([128, 1], F32, tag="dinv")
                    nc.vector.reciprocal(dinv[:], oe[:, DH:DH+1])
                    osc = strm.tile([128, DH], F32R, tag="osc")
                    nc.vector.tensor_scalar_mul(osc[:], oe[:, 0:DH], dinv[:])
                    ot_ps = pst.tile([DH, 128], F32R, tag="tp64")
                    nc.tensor.transpose(ot_ps[:], osc[:], ident_r[:])
                    if h == 0 and j == 0:
                        otb = otp_pool.tile([128, 2, N], F32R, tag="otb")
                    pb = (h % 2) * 64
                    nc.scalar.copy(otb[pb:pb+DH, h // 2, j*128:(j+1)*128], ot_ps[:])

            # ---- phase 3: output projection ----
            if LIMIT != "all":
                for j in range(NT):
                    y_z = strm.tile([128, 1024], F32, tag="ys", bufs=2)
                    nc.vector.memset(y_z[:], 0.0)
                    nc.sync.dma_start(y.ap()[j*128:(j+1)*128, :], y_z[:])
            # Each matmul contracts a head PAIR (K=128): otb chunk ch stacks
            # heads 2ch (rows 0:64) and 2ch+1 (rows 64:128); wo_r stacks the
            # matching Wo rows.  All operands at base partition 0 — mixing
            # base partitions inside one PSUM accumulation group is fatal.
            for j in range(NT if LIMIT == "all" else 0):
                y_ps = [psc.tile([128, 512], F32, tag=f"ctx{nb}", name=f"yps{nb}") for nb in range(2)]
                for nb in range(2):
                    for ch in range(2):
                        nc.tensor.matmul(y_ps[nb][:],
                                         otb[:, ch, j*128:(j+1)*128],
                                         wo_r[:, ch*1024 + nb*512:
                                              ch*1024 + nb*512 + 512],
                                         start=(ch == 0), stop=(ch == 1))
                y_s = strm.tile([128, 1024], F32, tag="ys", bufs=2)
                for nb in range(2):
                    nc.vector.tensor_copy(y_s[:, nb*512:(nb+1)*512], y_ps[nb][:])
                nc.sync.dma_start(y.ap()[j*128:(j+1)*128, :], y_s[:])

    nc.compile()
    return nc


_prog = None


def _build_in_maps(inputs):
    return _make_in_maps(**inputs)


def _make_in_maps(x, Wq, Wk, Wv, Wo, bo, proj):
    x = np.asarray(x, np.float32)
    projc = np.ascontiguousarray(CNORM * np.asarray(proj, np.float32).T)
    identm = np.eye(128, dtype=np.float32)
    xTb = [np.ascontiguousarray(x[b].T) for b in range(B)]
    in_maps = []
    for c in range(8):
        b, g = c // 4, c % 4
        hs, he = g * 256, g * 256 + 256
        woT = np.asarray(Wo, np.float32)[:, hs:he].T          # [256, 1024]
        woP = np.concatenate([woT[:128], woT[128:]], axis=1)  # [128, 2048]
        in_maps.append({
            "xT": xTb[b],
            "wqT": np.ascontiguousarray(np.asarray(Wq, np.float32)[hs:he].T),
            "wkT": np.ascontiguousarray(np.asarray(Wk, np.float32)[hs:he].T),
            "wvT": np.ascontiguousarray(np.asarray(Wv, np.float32)[hs:he].T),
            "woP": np.ascontiguousarray(woP),
            "projc": projc,
            "ident": identm,
        })
    return in_maps


def kernel(x, Wq, Wk, Wv, Wo, bo, proj):
    global _prog
    if _prog is None:
        _prog = build()
    in_maps = _make_in_maps(x, Wq, Wk, Wv, Wo, bo, proj)
    res = run_bass_kernel_spmd(_prog, in_maps, core_ids=list(range(8)))
    out = np.zeros((B, N, D), np.float32)
    for c in range(8):
        out[c // 4] += res.results[c]["y"]
    out += np.asarray(bo, np.float32)[None, None, :]
    return out



# revision 9
# speedup vs baseline: 2.1521x; 2.1521x over previous
"""Performer (FAVOR+) linear attention on 8 TRN2 NeuronCores.

Sharding: core c handles batch b=c//4 and head group g=c%4 (4 of 16 heads).
Host converts inputs to bf16, sums the 4 per-batch partials and adds bias.

v2 design (vs fp32r baseline at 1.385 ms):
 - all PE operands bf16 (1 cycle/col at any free dim, FWL weight loads);
   end-to-end error vs f32 reference ~7e-3 (measured in numpy).
 - qkv stays resident in SBUF (no DRAM round-trip).
 - context accumulated transposed (ctxT = vx.T @ kp): stationary vx,
   moving kp [n,267] -> one 111ns matmul per tile instead of 3 LDW-bound
   chunk matmuls.
 - output accumulated transposed (oeT = ctx.T @ qp.T): stationary ctx
   chunks, moving qpT 512 wide.
 - eps floor handled exactly via rank-1 corrections (matmul with ones /
   K=1 outer products); q-side rowmax and k-side global max over the raw
   dash match the reference eps semantics exactly.
 - per-head D row scaled via broadcast matmul (E2 @ D) + one DVE mult.
"""
import sys
sys.path.insert(0, '/opt/trn_rl_repo')

import numpy as np
import concourse.bass as bass
import concourse.bacc as bacc
import concourse.tile as tile
from concourse import mybir
from concourse.bass_utils import run_bass_kernel_spmd

F32 = mybir.dt.float32
F32R = mybir.dt.float32r
BF16 = mybir.dt.bfloat16
AX = mybir.AxisListType.X
AF = mybir.ActivationFunctionType
ALU = mybir.AluOpType

B, N, D = 2, 4096, 1024
H, DH, M = 16, 64, 266          # heads, dim_head, nb_features
HPC = 4                         # heads per core
EPS = 1e-4
CNORM = DH ** -0.25
RATIO = M ** -0.5
LNR = float(np.log(RATIO))
RE = float(RATIO * EPS)
NT = N // 128                   # 32 n-tiles
NB = N // 512                   # 8 n-blocks
MCH = [(0, 128), (128, 128), (256, 10)]   # m-chunks of 266


def build():
    nc = bacc.Bacc("TRN2", target_bir_lowering=False, debug=False)

    xT = nc.dram_tensor("xT", [D, N], BF16, kind="ExternalInput")
    wqkT = nc.dram_tensor("wqkT", [D, 512], BF16, kind="ExternalInput")
    wvT = nc.dram_tensor("wvT", [D, 256], BF16, kind="ExternalInput")
    woP = nc.dram_tensor("woP", [128, 2048], BF16, kind="ExternalInput")
    projc = nc.dram_tensor("projc", [DH, M], BF16, kind="ExternalInput")
    ident = nc.dram_tensor("ident", [128, 128], BF16, kind="ExternalInput")
    identf = nc.dram_tensor("identf", [128, 128], F32, kind="ExternalInput")
    e2d = nc.dram_tensor("e2d", [2, 128], F32, kind="ExternalInput")
    y = nc.dram_tensor("y", [N, D], BF16, kind="ExternalOutput")

    with tile.TileContext(nc) as tc:
        with tc.tile_pool(name="const", bufs=1) as cpool, \
             tc.tile_pool(name="big", bufs=1) as big, \
             tc.tile_pool(name="strm", bufs=3) as strm, \
             tc.tile_pool(name="sml", bufs=4) as sml, \
             tc.tile_pool(name="psA", bufs=2, space="PSUM") as psA, \
             tc.tile_pool(name="psV", bufs=2, space="PSUM") as psV, \
             tc.tile_pool(name="psD", bufs=1, space="PSUM") as psD, \
             tc.tile_pool(name="psB", bufs=2, space="PSUM") as psB, \
             tc.tile_pool(name="psC", bufs=1, space="PSUM") as psC:

            # ---- constants / weights ----
            wqk = cpool.tile([128, 8, 512], BF16, tag="wqk")
            nc.sync.dma_start(wqk[:], wqkT.ap().rearrange("(c p) n -> p c n", p=128))
            wv = cpool.tile([128, 8, 256], BF16, tag="wv")
            nc.sync.dma_start(wv[:], wvT.ap().rearrange("(c p) n -> p c n", p=128))
            wo = cpool.tile([128, 2048], BF16, tag="wo")
            nc.sync.dma_start(wo[:], woP.ap())
            pj = cpool.tile([DH, M], BF16, tag="pj")
            nc.sync.dma_start(pj[:], projc.ap())
            identb = cpool.tile([128, 128], BF16, tag="idb")
            nc.sync.dma_start(identb[:], ident.ap())
            identft = cpool.tile([128, 128], F32, tag="idf")
            nc.sync.dma_start(identft[:], identf.ap())
            ones1f = cpool.tile([1, 128], F32, tag="o1f")
            nc.vector.memset(ones1f[:], 1.0)
            ones1b = cpool.tile([1, 128], BF16, tag="o1b")
            nc.vector.memset(ones1b[:], 1.0)
            onesr512 = cpool.tile([1, 512], BF16, tag="o512")
            nc.vector.memset(onesr512[:], 1.0)
            onescol = cpool.tile([128, 1], BF16, tag="ocol")
            nc.vector.memset(onescol[:], 1.0)
            e2a = cpool.tile([1, 128], F32, tag="e2a")
            nc.sync.dma_start(e2a[:], e2d.ap()[0:1, :])
            e2b = cpool.tile([1, 128], F32, tag="e2b")
            nc.sync.dma_start(e2b[:], e2d.ap()[1:2, :])
            e2ar = cpool.tile([1, 128], F32R, tag="e2ar")
            nc.scalar.copy(e2ar[:], e2a[:])
            e2br = cpool.tile([1, 128], F32R, tag="e2br")
            nc.scalar.copy(e2br[:], e2b[:])

            qall = big.tile([128, NT, 256], BF16, tag="qall")
            kall = big.tile([128, NT, 256], BF16, tag="kall")
            vxall = big.tile([128, NT, HPC, 66], BF16, tag="vx")
            nc.vector.memset(vxall[:, :, :, 64:66], 1.0)
            dashkb = big.tile([128, NT, M], F32, tag="dashk")
            otbs = big.tile([128, 2, N], BF16, tag="otb")
            dpair = big.tile([1, N], F32R, tag="dpair")

            # ---- phase 1: QKV projections into SBUF ----
            for j in range(NT):
                xt = strm.tile([128, 8, 128], BF16, tag="xt")
                nc.sync.dma_start(
                    xt[:], xT.ap().rearrange("(c p) n -> p c n", p=128)[:, :, j*128:(j+1)*128])
                qk_ps = psA.tile([128, 512], F32, tag="dash", name=f"qk{j}")
                for c in range(8):
                    nc.tensor.matmul(qk_ps[:], xt[:, c, :], wqk[:, c, :],
                                     start=(c == 0), stop=(c == 7))
                v_ps = psV.tile([128, 256], F32, tag="pv", name=f"v{j}")
                for c in range(8):
                    nc.tensor.matmul(v_ps[:], xt[:, c, :], wv[:, c, :],
                                     start=(c == 0), stop=(c == 7))
                nc.any.tensor_copy(qall[:, j, :], qk_ps[:, 0:256])
                nc.any.tensor_copy(kall[:, j, :], qk_ps[:, 256:512])
                nc.any.tensor_copy(vxall[:, j, :, 0:64],
                                   v_ps[:].rearrange("p (h e) -> p h e", e=64))

            # ---- per-head chain ----
            for h in range(HPC):
                hs = h * 64
                # K1: dash_k tiles + row max + diag
                rmaxb = sml.tile([128, NT], F32, tag="rmaxb", bufs=2)
                diagkb = sml.tile([128, NT], F32, tag="diagkb", bufs=2)
                for j in range(NT):
                    sqj = strm.tile([128, DH], BF16, tag="sqj", bufs=2)
                    nc.scalar.activation(sqj[:], kall[:, j, hs:hs+64], AF.Square,
                                         accum_out=diagkb[:, j:j+1])
                    kt_ps = psV.tile([DH, 128], BF16, tag="pv", name=f"kt{h}_{j}")
                    nc.tensor.transpose(kt_ps[:], kall[:, j, hs:hs+64], identb[:])
                    kt = strm.tile([DH, 128], BF16, tag="kt")
                    nc.any.tensor_copy(kt[:], kt_ps[:])
                    dash_ps = psA.tile([128, M], F32, tag="dash", name=f"dk{h}_{j}")
                    nc.tensor.matmul(dash_ps[:], kt[:], pj[:], start=True, stop=True)
                    nc.vector.reduce_max(rmaxb[:, j:j+1], dash_ps[:], axis=AX)
                    nc.any.tensor_copy(dashkb[:, j, :], dash_ps[:])

                # global max of raw dash_k -> per-partition bias column
                gmax = sml.tile([128, 1], F32, tag="gmax")
                nc.vector.reduce_max(gmax[:], rmaxb[:], axis=AX)
                gm_ps = psV.tile([1, 128], F32, tag="pv", name=f"gm{h}")
                nc.tensor.transpose(gm_ps[:], gmax[:], identft[:])
                gmrow = sml.tile([1, 128], F32, tag="gmrow")
                nc.any.tensor_copy(gmrow[:], gm_ps[:])
                mk = sml.tile([1, 1], F32, tag="mk")
                nc.vector.reduce_max(mk[:], gmrow[:], axis=AX)
                mk_ps = psV.tile([128, 1], F32, tag="pv", name=f"mkb{h}")
                nc.tensor.matmul(mk_ps[:], ones1f[:], mk[:], start=True, stop=True)
                mkl = sml.tile([128, 1], F32, tag="mkl")
                nc.vector.tensor_scalar(mkl[:], mk_ps[:], -1.0, LNR,
                                        op0=ALU.mult, op1=ALU.add)
                biaskb = sml.tile([128, NT], F32, tag="biaskb", bufs=2)
                nc.vector.tensor_scalar(biaskb[:], diagkb[:],
                                        -0.5 * CNORM * CNORM, mkl[:],
                                        op0=ALU.mult, op1=ALU.add)

                # K2: kp = exp(dash - diag - mk + lnr); ctxT accumulation
                ctxT_ps = psD.tile([66, M + 1], F32, tag="ctxT", name=f"ctxT{h}")
                for j in range(NT):
                    kp = strm.tile([128, M + 1], BF16, tag="kp")
                    nc.scalar.activation(kp[:, 1:M+1], dashkb[:, j, :], AF.Exp,
                                         bias=biaskb[:, j:j+1], scale=1.0)
                    nc.vector.memset(kp[:, 0:1], 1.0)
                    nc.tensor.matmul(ctxT_ps[:], vxall[:, j, h, :], kp[:],
                                     start=(j == 0), stop=(j == NT - 1))

                # ctx chunks [m,66] with eps correction (+ RE * colsum_vx per col)
                ctxs = strm.tile([66, M + 1], BF16, tag="ctxs", bufs=2)
                nc.any.tensor_copy(ctxs[:], ctxT_ps[:])
                cv_ps = psV.tile([1, 66], BF16, tag="pv", name=f"cv{h}")
                nc.tensor.transpose(cv_ps[:], ctxs[:, 0:1], identb[0:66, 0:66])
                epsrow = sml.tile([1, 66], BF16, tag="epsrow", bufs=2)
                nc.scalar.mul(epsrow[:], cv_ps[:], RE)
                ep_ps = psV.tile([128, 66], F32, tag="pv", name=f"ep{h}")
                nc.tensor.matmul(ep_ps[:], ones1b[:], epsrow[:], start=True, stop=True)
                epsbc = sml.tile([128, 66], F32, tag="epsbc", bufs=2)
                nc.any.tensor_copy(epsbc[:], ep_ps[:])
                ctxc = strm.tile([128, 3, 66], BF16, tag="ctxc", bufs=2)
                for mc, (off, w) in enumerate(MCH):
                    cc_ps = psV.tile([128, 128], BF16, tag="pv", name=f"cc{h}_{mc}")
                    nc.tensor.transpose(cc_ps[0:w, 0:66], ctxs[:, 1+off:1+off+w],
                                        identb[0:66, 0:66])
                    nc.vector.tensor_add(ctxc[0:w, mc, :], cc_ps[0:w, 0:66],
                                         epsbc[0:w, :])
                # colsum of corrected ctx (for the q-side eps term)
                csc_ps = psV.tile([1, 66], F32, tag="pv", name=f"csc{h}")
                for mc, (off, w) in enumerate(MCH):
                    nc.tensor.matmul(csc_ps[:], onescol[0:w, :], ctxc[0:w, mc, :],
                                     start=(mc == 0), stop=(mc == 2))
                cscrow = sml.tile([1, 66], BF16, tag="cscrow", bufs=2)
                nc.scalar.mul(cscrow[:], csc_ps[:], RE)

                # Q pass: dash, rowmax, exp, transpose, oeT blocks
                qptb = None
                for j in range(NT):
                    sqq = strm.tile([128, DH], BF16, tag="sqj", bufs=2)
                    diagq = sml.tile([128, 1], F32, tag="diagq")
                    nc.scalar.activation(sqq[:], qall[:, j, hs:hs+64], AF.Square,
                                         accum_out=diagq[:])
                    qt_ps = psV.tile([DH, 128], BF16, tag="pv", name=f"qt{h}_{j}")
                    nc.tensor.transpose(qt_ps[:], qall[:, j, hs:hs+64], identb[:])
                    qt = strm.tile([DH, 128], BF16, tag="kt")
                    nc.any.tensor_copy(qt[:], qt_ps[:])
                    dq_ps = psA.tile([128, M], F32, tag="dash", name=f"dq{h}_{j}")
                    nc.tensor.matmul(dq_ps[:], qt[:], pj[:], start=True, stop=True)
                    rmaxq = sml.tile([128, 1], F32, tag="rmaxq")
                    nc.vector.reduce_max(rmaxq[:], dq_ps[:], axis=AX)
                    biasq = sml.tile([128, 1], F32, tag="biasq")
                    nc.vector.tensor_scalar(biasq[:], diagq[:],
                                            -0.5 * CNORM * CNORM, LNR,
                                            op0=ALU.mult, op1=ALU.add)
                    nc.vector.tensor_sub(biasq[:], biasq[:], rmaxq[:])
                    qp = strm.tile([128, M], BF16, tag="qp")
                    nc.scalar.activation(qp[:], dq_ps[:], AF.Exp,
                                         bias=biasq[:], scale=1.0)
                    jj, bb = j % 4, j // 4
                    if jj == 0:
                        qptb = strm.tile([128, 3, 512], BF16, tag="qptb", bufs=2)
                    qpt_ps = psB.tile([128, 384], BF16, tag="qpt")
                    for mc, (off, w) in enumerate(MCH):
                        nc.tensor.transpose(qpt_ps[0:w, mc*128:(mc+1)*128],
                                            qp[:, off:off+w], identb[:])
                    nc.any.tensor_copy(
                        qptb[:, 0:2, jj*128:(jj+1)*128],
                        qpt_ps[:, 0:256].rearrange("p (c n) -> p c n", n=128))
                    nc.any.tensor_copy(qptb[0:10, 2, jj*128:(jj+1)*128],
                                       qpt_ps[0:10, 256:384])
                    if jj == 3:
                        oeT_ps = psC.tile([66, 512], F32, tag="oeT", name=f"oe{h}_{bb}")
                        for mc, (off, w) in enumerate(MCH):
                            nc.tensor.matmul(oeT_ps[:], ctxc[0:w, mc, :],
                                             qptb[0:w, mc, :],
                                             start=(mc == 0), stop=False)
                        nc.tensor.matmul(oeT_ps[:], cscrow[:], onesr512[:],
                                         start=False, stop=True)
                        pb, ch = (h % 2) * 64, h // 2
                        sl = slice(bb*512, (bb+1)*512)
                        nc.any.tensor_copy(
                            otbs[pb:pb+64, ch, sl], oeT_ps[0:64, :])
                        if h % 2 == 0:
                            nc.any.tensor_copy(dpair[:, sl], oeT_ps[64:65, :])
                        else:
                            # normalize both heads of the pair for this block
                            dr = sml.tile([1, 512], F32R, tag="dr", bufs=2)
                            nc.any.tensor_copy(dr[:], oeT_ps[64:65, :])
                            db_ps = psV.tile([128, 512], F32, tag="pv",
                                             name=f"db{h}_{bb}")
                            nc.tensor.matmul(db_ps[:], e2ar[:], dpair[:, sl],
                                             start=True, stop=False)
                            nc.tensor.matmul(db_ps[:], e2br[:], dr[:],
                                             start=False, stop=True)
                            dinvb = sml.tile([128, 512], F32, tag="dinvb", bufs=2)
                            nc.vector.reciprocal(dinvb[:], db_ps[:])
                            nc.vector.tensor_mul(otbs[:, ch, sl],
                                                 otbs[:, ch, sl], dinvb[:])

            # ---- phase 3: output projection ----
            for j in range(NT):
                y_ps = [psA.tile([128, 512], F32, tag="dash", name=f"y{j}_{nb}")
                        for nb in range(2)]  # noqa: name passed explicitly
                for nb in range(2):
                    for ch in range(2):
                        nc.tensor.matmul(y_ps[nb][:],
                                         otbs[:, ch, j*128:(j+1)*128],
                                         wo[:, ch*1024 + nb*512:
                                            ch*1024 + nb*512 + 512],
                                         start=(ch == 0), stop=(ch == 1))
                y_s = strm.tile([128, 1024], BF16, tag="ys", bufs=2)
                for nb in range(2):
                    nc.any.tensor_copy(y_s[:, nb*512:(nb+1)*512], y_ps[nb][:])
                nc.sync.dma_start(y.ap()[j*128:(j+1)*128, :], y_s[:])

    nc.compile()
    return nc


_prog = None


def _bf16(a):
    from ml_dtypes import bfloat16
    return np.ascontiguousarray(np.asarray(a, np.float32)).astype(bfloat16)


def _build_in_maps(inputs):
    return _make_in_maps(**inputs)


def _make_in_maps(x, Wq, Wk, Wv, Wo, bo, proj):
    x = np.asarray(x, np.float32)
    Wq = np.asarray(Wq, np.float32)
    Wk = np.asarray(Wk, np.float32)
    Wv = np.asarray(Wv, np.float32)
    Wo = np.asarray(Wo, np.float32)
    projc = np.ascontiguousarray(CNORM * np.asarray(proj, np.float32).T)
    identm = np.eye(128, dtype=np.float32)
    e2m = np.zeros((2, 128), np.float32)
    e2m[0, 0:64] = 1.0
    e2m[1, 64:128] = 1.0
    xTb = [np.ascontiguousarray(x[b].T) for b in range(B)]
    in_maps = []
    for c in range(8):
        b, g = c // 4, c % 4
        hs, he = g * 256, g * 256 + 256
        woT = Wo[:, hs:he].T                                   # [256, 1024]
        woP = np.concatenate([woT[:128], woT[128:]], axis=1)   # [128, 2048]
        wqkT = np.concatenate([Wq[hs:he].T, Wk[hs:he].T], axis=1)  # [1024, 512]
        in_maps.append({
            "xT": _bf16(xTb[b]),
            "wqkT": _bf16(wqkT),
            "wvT": _bf16(Wv[hs:he].T),
            "woP": _bf16(woP),
            "projc": _bf16(projc),
            "ident": _bf16(identm),
            "identf": identm,
            "e2d": e2m,
        })
    return in_maps


def kernel(x, Wq, Wk, Wv, Wo, bo, proj):
    global _prog
    if _prog is None:
        _prog = build()
    in_maps = _make_in_maps(x, Wq, Wk, Wv, Wo, bo, proj)
    res = run_bass_kernel_spmd(_prog, in_maps, core_ids=list(range(8)))
    out = np.zeros((B, N, D), np.float32)
    for c in range(8):
        out[c // 4] += np.asarray(res.results[c]["y"], np.float32)
    out += np.asarray(bo, np.float32)[None, None, :]
    return out


# revision 15
# speedup vs baseline: 2.4511x; 1.1389x over previous
"""Performer (FAVOR+) linear attention on 8 TRN2 NeuronCores.

Sharding: core c handles batch b=c//4 and head group g=c%4 (4 of 16 heads).
Host converts inputs to bf16, sums the 4 per-batch partials and adds bias.

v2 design (vs fp32r baseline at 1.385 ms):
 - all PE operands bf16 (1 cycle/col at any free dim, FWL weight loads);
   end-to-end error vs f32 reference ~7e-3 (measured in numpy).
 - qkv stays resident in SBUF (no DRAM round-trip).
 - context accumulated transposed (ctxT = vx.T @ kp): stationary vx,
   moving kp [n,267] -> one 111ns matmul per tile instead of 3 LDW-bound
   chunk matmuls.
 - output accumulated transposed (oeT = ctx.T @ qp.T): stationary ctx
   chunks, moving qpT 512 wide.
 - eps floor handled exactly via rank-1 corrections (matmul with ones /
   K=1 outer products); q-side rowmax and k-side global max over the raw
   dash match the reference eps semantics exactly.
 - per-head D row scaled via broadcast matmul (E2 @ D) + one DVE mult.
"""
import sys
sys.path.insert(0, '/opt/trn_rl_repo')

import numpy as np
import concourse.bass as bass
import concourse.bacc as bacc
import concourse.tile as tile
from concourse import mybir
from concourse.bass_utils import run_bass_kernel_spmd

F32 = mybir.dt.float32
F32R = mybir.dt.float32r
BF16 = mybir.dt.bfloat16
AX = mybir.AxisListType.X
AF = mybir.ActivationFunctionType
ALU = mybir.AluOpType

B, N, D = 2, 4096, 1024
H, DH, M = 16, 64, 266          # heads, dim_head, nb_features
HPC = 4                         # heads per core
EPS = 1e-4
CNORM = DH ** -0.25
RATIO = M ** -0.5
LNR = float(np.log(RATIO))
RE = float(RATIO * EPS)
NT = N // 128                   # 32 n-tiles
NB = N // 512                   # 8 n-blocks
MCH = [(0, 128), (128, 128), (256, 10)]   # m-chunks of 266
CN2 = float(np.sqrt(0.5) * CNORM)         # scale so Square-accum yields 0.5*c^2*sum(k^2)


def build():
    nc = bacc.Bacc("TRN2", target_bir_lowering=False, debug=False)

    xT = nc.dram_tensor("xT", [D, N], BF16, kind="ExternalInput")
    wqkT = nc.dram_tensor("wqkT", [D, 512], BF16, kind="ExternalInput")
    wvT = nc.dram_tensor("wvT", [D, 256], BF16, kind="ExternalInput")
    woP = nc.dram_tensor("woP", [128, 2048], BF16, kind="ExternalInput")
    projc = nc.dram_tensor("projc", [DH, M], BF16, kind="ExternalInput")
    ident = nc.dram_tensor("ident", [128, 128], BF16, kind="ExternalInput")
    identf = nc.dram_tensor("identf", [128, 128], F32, kind="ExternalInput")
    e2d = nc.dram_tensor("e2d", [2, 128], F32, kind="ExternalInput")
    y = nc.dram_tensor("y", [N, D], BF16, kind="ExternalOutput")

    with tile.TileContext(nc) as tc:
        with tc.tile_pool(name="const", bufs=1) as cpool, \
             tc.tile_pool(name="big", bufs=1) as big, \
             tc.tile_pool(name="strm", bufs=3) as strm, \
             tc.tile_pool(name="sml", bufs=4) as sml, \
             tc.tile_pool(name="psA", bufs=3, space="PSUM") as psA, \
             tc.tile_pool(name="psV", bufs=2, space="PSUM") as psV, \
             tc.tile_pool(name="psD", bufs=1, space="PSUM") as psD, \
             tc.tile_pool(name="psC", bufs=2, space="PSUM") as psC:

            # ---- constants / weights ----
            wqk = cpool.tile([128, 8, 512], BF16, tag="wqk")
            nc.sync.dma_start(wqk[:], wqkT.ap().rearrange("(c p) n -> p c n", p=128))
            wv = cpool.tile([128, 8, 256], BF16, tag="wv")
            nc.sync.dma_start(wv[:], wvT.ap().rearrange("(c p) n -> p c n", p=128))
            wo = cpool.tile([128, 2048], BF16, tag="wo")
            nc.sync.dma_start(wo[:], woP.ap())
            pj = cpool.tile([128, M], BF16, tag="pj")
            nc.sync.dma_start(pj[0:64, :], projc.ap())
            nc.sync.dma_start(pj[64:128, :], projc.ap())
            identb = cpool.tile([128, 128], BF16, tag="idb")
            nc.sync.dma_start(identb[:], ident.ap())
            identft = cpool.tile([128, 128], F32, tag="idf")
            nc.sync.dma_start(identft[:], identf.ap())
            ones1f = cpool.tile([1, 128], F32, tag="o1f")
            nc.vector.memset(ones1f[:], 1.0)
            ones1b = cpool.tile([1, 128], BF16, tag="o1b")
            nc.vector.memset(ones1b[:], 1.0)
            onesr512 = cpool.tile([1, 512], BF16, tag="o512")
            nc.vector.memset(onesr512[:], 1.0)
            onescol = cpool.tile([128, 1], BF16, tag="ocol")
            nc.vector.memset(onescol[:], 1.0)
            e2a = cpool.tile([1, 128], F32, tag="e2a")
            nc.sync.dma_start(e2a[:], e2d.ap()[0:1, :])
            e2b = cpool.tile([1, 128], F32, tag="e2b")
            nc.sync.dma_start(e2b[:], e2d.ap()[1:2, :])
            e2ar = cpool.tile([1, 128], F32R, tag="e2ar")
            nc.scalar.copy(e2ar[:], e2a[:])
            e2br = cpool.tile([1, 128], F32R, tag="e2br")
            nc.scalar.copy(e2br[:], e2b[:])

            qall = big.tile([128, NT, 256], BF16, tag="qall")
            kall = big.tile([128, NT, 256], BF16, tag="kall")
            qTall = big.tile([128, NT, 2, 128], BF16, tag="qTall")
            kTall = big.tile([128, NT, 2, 128], BF16, tag="kTall")
            vxall = big.tile([128, NT, HPC, 66], BF16, tag="vx")
            nc.vector.memset(vxall[:, :, :, 64:66], 1.0)
            dashkb = big.tile([128, NT, M], F32, tag="dashk")
            otbs = big.tile([128, 2, N], BF16, tag="otb")
            dpair = big.tile([1, N], F32R, tag="dpair")

            # ---- phase 1: QKV projections into SBUF ----
            for j in range(NT):
                xt = strm.tile([128, 8, 128], BF16, tag="xt")
                nc.sync.dma_start(
                    xt[:], xT.ap().rearrange("(c p) n -> p c n", p=128)[:, :, j*128:(j+1)*128])
                qk_ps = psA.tile([128, 512], F32, tag="dash", name=f"qk{j}")
                for c in range(8):
                    nc.tensor.matmul(qk_ps[:], xt[:, c, :], wqk[:, c, :],
                                     start=(c == 0), stop=(c == 7))
                v_ps = psV.tile([128, 256], F32, tag="pv", name=f"v{j}")
                for c in range(8):
                    nc.tensor.matmul(v_ps[:], xt[:, c, :], wv[:, c, :],
                                     start=(c == 0), stop=(c == 7))
                nc.any.tensor_copy(qall[:, j, :], qk_ps[:, 0:256])
                nc.any.tensor_copy(kall[:, j, :], qk_ps[:, 256:512])
                nc.any.tensor_copy(vxall[:, j, :, 0:64],
                                   v_ps[:].rearrange("p (h e) -> p h e", e=64))
                for g in range(2):
                    qt_ps = psV.tile([128, 128], BF16, tag="pv", name=f"qt{j}_{g}")
                    nc.tensor.transpose(qt_ps[:], qall[:, j, g*128:(g+1)*128],
                                        identb[:])
                    nc.any.tensor_copy(qTall[:, j, g, :], qt_ps[:])
                    kt_ps = psV.tile([128, 128], BF16, tag="pv", name=f"kt{j}_{g}")
                    nc.tensor.transpose(kt_ps[:], kall[:, j, g*128:(g+1)*128],
                                        identb[:])
                    nc.any.tensor_copy(kTall[:, j, g, :], kt_ps[:])

            # ---- per-head chain ----
            for h in range(HPC):
                hs = h * 64
                # K1: dash_k tiles + row max + diag
                rmaxb = sml.tile([128, NT], F32, tag="rmaxb", bufs=2)
                diagkb = sml.tile([128, NT], F32, tag="diagkb", bufs=2)
                pb, ch = (h % 2) * 64, h // 2
                for j in range(NT):
                    sqj = strm.tile([128, DH], BF16, tag="sqj", bufs=2)
                    nc.scalar.activation(sqj[:], kall[:, j, hs:hs+64], AF.Square,
                                         scale=CN2, accum_out=diagkb[:, j:j+1])
                    dash_ps = psA.tile([128, M], F32, tag="dash", name=f"dk{h}_{j}")
                    nc.tensor.matmul(dash_ps[:], kTall[pb:pb+64, j, ch, :],
                                     pj[pb:pb+64, :], start=True, stop=True)
                    nc.any.tensor_copy(dashkb[:, j, :], dash_ps[:])
                    nc.vector.reduce_max(rmaxb[:, j:j+1], dashkb[:, j, :], axis=AX)

                # global max of raw dash_k -> per-partition bias column
                gmax = sml.tile([128, 1], F32, tag="gmax")
                nc.vector.reduce_max(gmax[:], rmaxb[:], axis=AX)
                gm_ps = psV.tile([1, 128], F32, tag="pv", name=f"gm{h}")
                nc.tensor.transpose(gm_ps[:], gmax[:], identft[:])
                gmrow = sml.tile([1, 128], F32, tag="gmrow")
                nc.any.tensor_copy(gmrow[:], gm_ps[:])
                mk = sml.tile([1, 1], F32, tag="mk")
                nc.vector.reduce_max(mk[:], gmrow[:], axis=AX)
                mk_ps = psV.tile([128, 1], F32, tag="pv", name=f"mkb{h}")
                nc.tensor.matmul(mk_ps[:], ones1f[:], mk[:], start=True, stop=True)
                mkl = sml.tile([128, 1], F32, tag="mkl")
                nc.vector.tensor_scalar(mkl[:], mk_ps[:], -1.0, LNR,
                                        op0=ALU.mult, op1=ALU.add)
                biaskb = sml.tile([128, NT], F32, tag="biaskb", bufs=2)
                nc.vector.tensor_scalar(biaskb[:], diagkb[:], -1.0, mkl[:],
                                        op0=ALU.mult, op1=ALU.add)

                # K2: kp = exp(dash - diag - mk + lnr); ctxT accumulation
                ctxT_ps = psD.tile([66, M + 1], F32, tag="ctxT", name=f"ctxT{h}")
                for j in range(NT):
                    kp = strm.tile([128, M + 1], BF16, tag="kp")
                    nc.scalar.activation(kp[:, 1:M+1], dashkb[:, j, :], AF.Exp,
                                         bias=biaskb[:, j:j+1], scale=1.0)
                    nc.vector.memset(kp[:, 0:1], 1.0)
                    nc.tensor.matmul(ctxT_ps[:], vxall[:, j, h, :], kp[:],
                                     start=(j == 0), stop=(j == NT - 1))

                # ctx chunks [m,66] with eps correction (+ RE * colsum_vx per col)
                ctxs = strm.tile([66, M + 1], BF16, tag="ctxs", bufs=2)
                nc.any.tensor_copy(ctxs[:], ctxT_ps[:])
                cv_ps = psV.tile([1, 66], BF16, tag="pv", name=f"cv{h}")
                nc.tensor.transpose(cv_ps[:], ctxs[:, 0:1], identb[0:66, 0:66])
                epsrow = sml.tile([1, 66], BF16, tag="epsrow", bufs=2)
                nc.scalar.mul(epsrow[:], cv_ps[:], RE)
                ep_ps = psV.tile([128, 66], F32, tag="pv", name=f"ep{h}")
                nc.tensor.matmul(ep_ps[:], ones1b[:], epsrow[:], start=True, stop=True)
                epsbc = sml.tile([128, 66], F32, tag="epsbc", bufs=2)
                nc.any.tensor_copy(epsbc[:], ep_ps[:])
                ctxc = strm.tile([128, 3, 66], BF16, tag="ctxc", bufs=2)
                for mc, (off, w) in enumerate(MCH):
                    cc_ps = psV.tile([128, 128], BF16, tag="pv", name=f"cc{h}_{mc}")
                    nc.tensor.transpose(cc_ps[0:w, 0:66], ctxs[:, 1+off:1+off+w],
                                        identb[0:66, 0:66])
                    nc.vector.tensor_add(ctxc[0:w, mc, :], cc_ps[0:w, 0:66],
                                         epsbc[0:w, :])
                # colsum of corrected ctx (for the q-side eps term)
                csc_ps = psV.tile([1, 66], F32, tag="pv", name=f"csc{h}")
                for mc, (off, w) in enumerate(MCH):
                    nc.tensor.matmul(csc_ps[:], onescol[0:w, :], ctxc[0:w, mc, :],
                                     start=(mc == 0), stop=(mc == 2))
                cscrow = sml.tile([1, 66], BF16, tag="cscrow", bufs=2)
                nc.scalar.mul(cscrow[:], csc_ps[:], RE)

                # Q pass: dash, rowmax, exp, transpose, oeT blocks
                qptb = None
                for j in range(NT):
                    sqq = strm.tile([128, DH], BF16, tag="sqj", bufs=2)
                    diagq = sml.tile([128, 1], F32, tag="diagq")
                    nc.scalar.activation(sqq[:], qall[:, j, hs:hs+64], AF.Square,
                                         scale=CN2, accum_out=diagq[:])
                    dq_ps = psA.tile([128, M], F32, tag="dash", name=f"dq{h}_{j}")
                    nc.tensor.matmul(dq_ps[:], qTall[pb:pb+64, j, ch, :],
                                     pj[pb:pb+64, :], start=True, stop=True)
                    rmaxq = sml.tile([128, 1], F32, tag="rmaxq")
                    nc.vector.reduce_max(rmaxq[:], dq_ps[:], axis=AX)
                    biasq = sml.tile([128, 1], F32, tag="biasq")
                    nc.vector.tensor_scalar(biasq[:], diagq[:], -1.0, LNR,
                                            op0=ALU.mult, op1=ALU.add)
                    nc.vector.tensor_sub(biasq[:], biasq[:], rmaxq[:])
                    qp = strm.tile([128, 384], BF16, tag="qp")
                    nc.scalar.activation(qp[:, 0:M], dq_ps[:], AF.Exp,
                                         bias=biasq[:], scale=1.0)
                    jj, bb = j % 4, j // 4
                    if jj == 0:
                        qptb = strm.tile([128, 3, 512], BF16, tag="qptb", bufs=2)
                    qpt_ps = psV.tile([128, 384], BF16, tag="pv", name=f"qp{h}_{j}")
                    for mc, (off, w) in enumerate(MCH):
                        nc.tensor.transpose(qpt_ps[0:w, mc*128:(mc+1)*128],
                                            qp[:, off:off+w], identb[:])
                    nc.any.tensor_copy(
                        qptb[:, 0:2, jj*128:(jj+1)*128],
                        qpt_ps[:, 0:256].rearrange("p (c n) -> p c n", n=128))
                    nc.any.tensor_copy(qptb[0:10, 2, jj*128:(jj+1)*128],
                                       qpt_ps[0:10, 256:384])
                    if jj == 3:
                        oeT_ps = psC.tile([66, 512], F32, tag="oeT", name=f"oe{h}_{bb}")
                        for mc, (off, w) in enumerate(MCH):
                            nc.tensor.matmul(oeT_ps[:], ctxc[0:w, mc, :],
                                             qptb[0:w, mc, :],
                                             start=(mc == 0), stop=False)
                        nc.tensor.matmul(oeT_ps[:], cscrow[:], onesr512[:],
                                         start=False, stop=True)
                        pb, ch = (h % 2) * 64, h // 2
                        sl = slice(bb*512, (bb+1)*512)
                        nc.any.tensor_copy(
                            otbs[pb:pb+64, ch, sl], oeT_ps[0:64, :])
                        if h % 2 == 0:
                            nc.any.tensor_copy(dpair[:, sl], oeT_ps[64:65, :])
                        else:
                            # normalize both heads of the pair for this block
                            dr = sml.tile([1, 512], F32R, tag="dr", bufs=2)
                            nc.any.tensor_copy(dr[:], oeT_ps[64:65, :])
                            db_ps = psV.tile([128, 512], F32, tag="pv",
                                             name=f"db{h}_{bb}")
                            nc.tensor.matmul(db_ps[:], e2ar[:], dpair[:, sl],
                                             start=True, stop=False)
                            nc.tensor.matmul(db_ps[:], e2br[:], dr[:],
                                             start=False, stop=True)
                            dinvb = sml.tile([128, 512], F32, tag="dinvb", bufs=2)
                            nc.vector.reciprocal(dinvb[:], db_ps[:])
                            nc.vector.tensor_mul(otbs[:, ch, sl],
                                                 otbs[:, ch, sl], dinvb[:])

            # ---- phase 3: output projection ----
            for j in range(NT):
                y_ps = [psA.tile([128, 512], F32, tag="dash", name=f"y{j}_{nb}")
                        for nb in range(2)]  # noqa: name passed explicitly
                for nb in range(2):
                    for ch in range(2):
                        nc.tensor.matmul(y_ps[nb][:],
                                         otbs[:, ch, j*128:(j+1)*128],
                                         wo[:, ch*1024 + nb*512:
                                            ch*1024 + nb*512 + 512],
                                         start=(ch == 0), stop=(ch == 1))
                y_s = strm.tile([128, 1024], BF16, tag="ys", bufs=2)
                for nb in range(2):
                    nc.any.tensor_copy(y_s[:, nb*512:(nb+1)*512], y_ps[nb][:])
                nc.sync.dma_start(y.ap()[j*128:(j+1)*128, :], y_s[:])

    nc.compile()
    return nc


_prog = None


def _bf16(a):
    from ml_dtypes import bfloat16
    return np.ascontiguousarray(np.asarray(a, np.float32)).astype(bfloat16)


def _build_in_maps(inputs):
    return _make_in_maps(**inputs)


def _make_in_maps(x, Wq, Wk, Wv, Wo, bo, proj):
    x = np.asarray(x, np.float32)
    Wq = np.asarray(Wq, np.float32)
    Wk = np.asarray(Wk, np.float32)
    Wv = np.asarray(Wv, np.float32)
    Wo = np.asarray(Wo, np.float32)
    projc = np.ascontiguousarray(CNORM * np.asarray(proj, np.float32).T)
    identm = np.eye(128, dtype=np.float32)
    e2m = np.zeros((2, 128), np.float32)
    e2m[0, 0:64] = 1.0
    e2m[1, 64:128] = 1.0
    xTb = [np.ascontiguousarray(x[b].T) for b in range(B)]
    in_maps = []
    for c in range(8):
        b, g = c // 4, c % 4
        hs, he = g * 256, g * 256 + 256
        woT = Wo[:, hs:he].T                                   # [256, 1024]
        woP = np.concatenate([woT[:128], woT[128:]], axis=1)   # [128, 2048]
        wqkT = np.concatenate([Wq[hs:he].T, Wk[hs:he].T], axis=1)  # [1024, 512]
        in_maps.append({
            "xT": _bf16(xTb[b]),
            "wqkT": _bf16(wqkT),
            "wvT": _bf16(Wv[hs:he].T),
            "woP": _bf16(woP),
            "projc": _bf16(projc),
            "ident": _bf16(identm),
            "identf": identm,
            "e2d": e2m,
        })
    return in_maps


def kernel(x, Wq, Wk, Wv, Wo, bo, proj):
    global _prog
    if _prog is None:
        _prog = build()
    in_maps = _make_in_maps(x, Wq, Wk, Wv, Wo, bo, proj)
    res = run_bass_kernel_spmd(_prog, in_maps, core_ids=list(range(8)))
    out = np.zeros((B, N, D), np.float32)
    for c in range(8):
        out[c // 4] += np.asarray(res.results[c]["y"], np.float32)
    out += np.asarray(bo, np.float32)[None, None, :]
    return out
